# revision 7
# baseline (speedup 1.0000x reference)
"""Trainium2 Bass kernel: conv-encoder + InfoNCE contrastive loss (scalar out).

Sharding: 8-way data parallel. Core c encodes images
  [256c, 256c+256)  (anchors)   and   [2048+256c, 2048+256c+256)  (targets),
so anchor/target PAIRS are core-local (diagonal of the logits matrix needs no
cross-core indexing and the program is identical on every core: pure SPMD).
Targets' embeddings are AllGather'd (tiny: 2048x64 f32), each core computes its
256 rows of the 2048x2048 logits matrix + logsumexp, partial sums are
AllReduce'd, every core computes the same final scalar.

Encoder mapping:
  - obs is host-padded to 13x13 (zero border) and host-transposed to
    [84, B, 169] bf16, so each 3x3 conv = 9 PSUM-accumulated matmuls over
    shifted window access patterns (channels on partitions).
  - conv3 (stride 2) writes its relu output into a resident SBUF tensor X3 in
    (pos, chan-half, batch) order, which makes fc1's 72 K-chunks contiguous.
    fc1_w columns are host-permuted to match.
  - FC stack keeps features on partitions / batch on free dim. LayerNorm
    reduces over features (= partitions) with ones-vector matmuls; per-batch
    stats rows are broadcast back across partitions with a [1,128]-ones
    matmul. gamma/beta + SiLU are fused into one scalar-engine activation.
"""

import os
import sys

import numpy as np

try:
    import concourse.bass as bass  # noqa: F401
except Exception:  # pragma: no cover
    for _p in ("/opt/trn_rl_repo", os.path.expanduser("~/.axon_site/_ro/trn_rl_repo")):
        if os.path.isdir(_p) and _p not in sys.path:
            sys.path.insert(0, _p)
    import concourse.bass as bass

import ml_dtypes
import concourse.tile as tile
from concourse import bacc, mybir
from concourse.bass_utils import run_bass_kernel_spmd

F32 = mybir.dt.float32
BF16 = mybir.dt.bfloat16
AF = mybir.ActivationFunctionType
ALU = mybir.AluOpType

N_CORES = 8
B_CORE = 512          # images per core (full problem)
NB = 4                # images per conv batch-tile (nb*121 = 484 <= 512 PSUM)
LN_EPS = 1e-5
DIST_EPS = 1e-8

BF = ml_dtypes.bfloat16


# ----------------------------------------------------------------------------
# program builder
# ----------------------------------------------------------------------------

def build_program(B=B_CORE, nb=NB):
    assert B % nb == 0 and B % 2 == 0
    na = B // 2                 # anchors per core
    T = N_CORES * na            # total targets (== total anchors)
    nchunk = (na + 127) // 128  # anchor row chunks per core

    nc = bacc.Bacc("TRN2", target_bir_lowering=False, debug=False,
                   num_devices=N_CORES)

    dt_in = {}

    def din(name, shape, dtype):
        dt_in[name] = nc.dram_tensor(name, list(shape), dtype,
                                     kind="ExternalInput").ap()
        return dt_in[name]

    obs_t = din("obs_t", [84, B * 169], BF16)
    w1p = din("w1p", [84, 9 * 64], BF16)
    w2p = din("w2p", [64, 9 * 128], BF16)
    w3p = din("w3p", [128, 18 * 128], BF16)
    fc1wp = din("fc1wp", [128, 72 * 4 * 128], BF16)
    fc2wp = din("fc2wp", [128, 16 * 128], BF16)
    outwp = din("outwp", [128, 4 * 64], BF16)
    b1h = din("b1h", [64, 1], F32)
    b2h = din("b2h", [128, 1], F32)
    b3h = din("b3h", [128, 2], F32)
    fc1bh = din("fc1bh", [128, 4], F32)
    g1h = din("g1h", [128, 4], F32)
    be1h = din("be1h", [128, 4], F32)
    fc2bh = din("fc2bh", [128, 4], F32)
    g2h = din("g2h", [128, 4], F32)
    be2h = din("be2h", [128, 4], F32)
    outbh = din("outbh", [64, 1], F32)
    onesh = din("onesh", [128, 1], F32)
    onesrh = din("onesrh", [1, 128], F32)
    identh = din("identh", [1, 1], F32)
    epsh = din("epsh", [128, 2], F32)

    loss_out = nc.dram_tensor("loss_out", [1, 1], F32, kind="ExternalOutput").ap()

    C1 = 0.1 / T
    C2 = 0.01 / T

    with tile.TileContext(nc) as tc:
        with (
            tc.tile_pool(name="persist", bufs=1) as p_per,
            tc.tile_pool(name="wconst", bufs=1) as p_w,
        ):
            # ---- resident tensors ------------------------------------------
            X3 = p_per.tile([128, 2 * 36 * B], BF16, name="X3")
            embT = p_per.tile([64, B], F32, name="embT")

            # ---- load constants --------------------------------------------
            def ld(name, src):
                t = p_w.tile(list(src.shape), src.dtype, name=name)
                nc.sync.dma_start(t[:], src)
                return t

            w1s = ld("w1s", w1p)
            w2s = ld("w2s", w2p)
            w3s = ld("w3s", w3p)
            fc2ws = ld("fc2ws", fc2wp)
            outws = ld("outws", outwp)
            b1s = ld("b1s", b1h)
            b2s = ld("b2s", b2h)
            b3s = ld("b3s", b3h)
            fc1bs = ld("fc1bs", fc1bh)
            g1s = ld("g1s", g1h)
            be1s = ld("be1s", be1h)
            fc2bs = ld("fc2bs", fc2bh)
            g2s = ld("g2s", g2h)
            be2s = ld("be2s", be2h)
            outbs = ld("outbs", outbh)
            ones = ld("ones", onesh)
            onesr = ld("onesr", onesrh)
            ident = ld("ident", identh)
            epss = ld("epss", epsh)

            X3r = X3.rearrange("p (h q b) -> p h b q", h=2, q=36, b=B)

            # ================= conv stack ===================================
            with (
                tc.tile_pool(name="convsb", bufs=1) as p_c,
                tc.tile_pool(name="convps", space="PSUM", bufs=1) as p_cp,
            ):
                for it in range(B // nb):
                    x1 = p_c.tile([84, nb * 169], BF16, tag="x1", bufs=3,
                                  name="x1")
                    nc.sync.dma_start(
                        x1[:], obs_t[:, it * nb * 169:(it + 1) * nb * 169])
                    x1r = x1.rearrange("p (b h w) -> p b h w", h=13, w=13)

                    # conv1: 84 -> 64
                    ps1 = p_cp.tile([64, nb * 121], F32, tag="ps1", bufs=2,
                                    name="ps1")
                    for t in range(9):
                        dy, dx = divmod(t, 3)
                        nc.tensor.matmul(
                            ps1[:], w1s[:, t * 64:(t + 1) * 64],
                            x1r[:, :, dy:dy + 11, dx:dx + 11],
                            start=(t == 0), stop=(t == 8))

                    x2 = p_c.tile([64, nb * 169], BF16, tag="x2", bufs=2,
                                  name="x2")
                    x2r = x2.rearrange("p (b h w) -> p b h w", h=13, w=13)
                    nc.vector.memset(x2r[:, :, 0, :], 0.0)
                    nc.vector.memset(x2r[:, :, 12, :], 0.0)
                    nc.vector.memset(x2r[:, :, 1:12, 0:13:12], 0.0)
                    nc.scalar.activation(
                        x2r[:, :, 1:12, 1:12],
                        ps1.rearrange("p (b h w) -> p b h w", h=11, w=11),
                        AF.Relu, bias=b1s[:, 0:1])

                    # conv2: 64 -> 128
                    ps2 = p_cp.tile([128, nb * 121], F32, tag="ps2", bufs=2,
                                    name="ps2")
                    for t in range(9):
                        dy, dx = divmod(t, 3)
                        nc.tensor.matmul(
                            ps2[:], w2s[:, t * 128:(t + 1) * 128],
                            x2r[:, :, dy:dy + 11, dx:dx + 11],
                            start=(t == 0), stop=(t == 8))

                    x3 = p_c.tile([128, nb * 169], BF16, tag="x3", bufs=2,
                                  name="x3")
                    x3r = x3.rearrange("p (b h w) -> p b h w", h=13, w=13)
                    nc.vector.memset(x3r[:, :, 0, :], 0.0)
                    nc.vector.memset(x3r[:, :, 12, :], 0.0)
                    nc.vector.memset(x3r[:, :, 1:12, 0:13:12], 0.0)
                    nc.scalar.activation(
                        x3r[:, :, 1:12, 1:12],
                        ps2.rearrange("p (b h w) -> p b h w", h=11, w=11),
                        AF.Relu, bias=b2s[:, 0:1])

                    # conv3: 128 -> 256, stride 2 -> into resident X3
                    for h in range(2):
                        ps3 = p_cp.tile([128, nb * 36], F32, tag="ps3", bufs=2,
                                        name="ps3")
                        for t in range(9):
                            dy, dx = divmod(t, 3)
                            nc.tensor.matmul(
                                ps3[:],
                                w3s[:, (h * 9 + t) * 128:(h * 9 + t + 1) * 128],
                                x3r[:, :, dy:dy + 11:2, dx:dx + 11:2],
                                start=(t == 0), stop=(t == 8))
                        nc.scalar.activation(
                            X3r[:, h, it * nb:(it + 1) * nb, :],
                            ps3.rearrange("p (b q) -> p b q", q=36),
                            AF.Relu, bias=b3s[:, h:h + 1])

            # ================= fc stack =====================================
            with (
                tc.tile_pool(name="fcsb", bufs=1) as p_f,
                tc.tile_pool(name="fcps", space="PSUM", bufs=1) as p_fp,
            ):
                X3q = X3.rearrange("p (h q b) -> p h q b", h=2, q=36, b=B)

                def ln_layer(pre_chunks, bias_t, g_t, be_t, out_dt, nm):
                    """pre_chunks: list of 4 PSUM APs [128,B] (pre-LN).
                    Returns SBUF tile [128, 4*B] out_dt after LN+SiLU."""
                    hs = p_f.tile([128, 4 * B], F32, tag=f"hs{nm}", bufs=1,
                                  name=f"hs{nm}")
                    for m in range(4):
                        nc.scalar.activation(hs[:, m * B:(m + 1) * B],
                                             pre_chunks[m], AF.Identity,
                                             bias=bias_t[:, m:m + 1])
                    sq = p_f.tile([128, 4 * B], F32, tag="sq", bufs=1,
                                  name=f"sq{nm}")
                    nc.vector.tensor_mul(sq[:], hs[:], hs[:])
                    ps_sum = p_fp.tile([1, B], F32, tag="stat", bufs=2,
                                       name=f"sum{nm}")
                    for m in range(4):
                        nc.tensor.matmul(ps_sum[:], ones[:],
                                         hs[:, m * B:(m + 1) * B],
                                         start=(m == 0), stop=(m == 3),
                                         skip_group_check=True)
                    ps_sq = p_fp.tile([1, B], F32, tag="stat", bufs=2,
                                      name=f"ssq{nm}")
                    for m in range(4):
                        nc.tensor.matmul(ps_sq[:], ones[:],
                                         sq[:, m * B:(m + 1) * B],
                                         start=(m == 0), stop=(m == 3),
                                         skip_group_check=True)
                    rows = p_f.tile([1, 4 * B], F32, tag="rows", bufs=2,
                                    name=f"rows{nm}")
                    mean = rows[:, 0:B]
                    var = rows[:, B:2 * B]
                    sd = rows[:, 2 * B:3 * B]
                    rstd = rows[:, 3 * B:4 * B]
                    nc.vector.tensor_scalar_mul(mean, ps_sum[:], 1.0 / 512.0)
                    nc.vector.tensor_scalar_mul(var, ps_sq[:], 1.0 / 512.0)
                    msq = p_f.tile([1, B], F32, tag="msq", bufs=2,
                                   name=f"msq{nm}")
                    nc.vector.tensor_mul(msq[:], mean, mean)
                    nc.vector.tensor_sub(var, var, msq[:])
                    nc.scalar.activation(sd, var, AF.Sqrt, bias=epss[:1, 0:1])
                    nc.vector.reciprocal(rstd, sd)
                    negmr = p_f.tile([1, B], F32, tag="negmr", bufs=2,
                                     name=f"negmr{nm}")
                    nc.vector.scalar_tensor_tensor(
                        negmr[:], mean, -1.0, rstd, op0=ALU.mult, op1=ALU.mult)
                    # broadcast rows across partitions via ones-matmul
                    ps_r = p_fp.tile([128, B], F32, tag="bc", bufs=2,
                                     name=f"bcr{nm}")
                    nc.tensor.matmul(ps_r[:], onesr[:], rstd, start=True,
                                     stop=True)
                    rstdB = p_f.tile([128, B], F32, tag="rstdB", bufs=1,
                                     name=f"rstdB{nm}")
                    nc.vector.tensor_copy(rstdB[:], ps_r[:])
                    ps_n = p_fp.tile([128, B], F32, tag="bc", bufs=2,
                                     name=f"bcn{nm}")
                    nc.tensor.matmul(ps_n[:], onesr[:], negmr[:], start=True,
                                     stop=True)
                    negmrB = p_f.tile([128, B], F32, tag="negmrB", bufs=1,
                                      name=f"negmrB{nm}")
                    nc.vector.tensor_copy(negmrB[:], ps_n[:])

                    ha = p_f.tile([128, 4 * B], out_dt, tag=f"ha{nm}", bufs=1,
                                  name=f"ha{nm}")
                    for m in range(4):
                        tt = p_f.tile([128, B], F32, tag="scr", bufs=2,
                                      name=f"scr{nm}_{m}")
                        nc.vector.tensor_mul(tt[:], hs[:, m * B:(m + 1) * B],
                                             rstdB[:])
                        nc.vector.tensor_add(tt[:], tt[:], negmrB[:])
                        # silu(z) = z * sigmoid(z), z = g*norm + beta
                        z = p_f.tile([128, B], F32, tag="z", bufs=2,
                                     name=f"z{nm}_{m}")
                        nc.vector.tensor_scalar(z[:], tt[:],
                                                g_t[:, m:m + 1],
                                                be_t[:, m:m + 1],
                                                op0=ALU.mult, op1=ALU.add)
                        sg = p_f.tile([128, B], F32, tag="sg", bufs=2,
                                      name=f"sg{nm}_{m}")
                        nc.scalar.activation(sg[:], z[:], AF.Sigmoid)
                        nc.vector.tensor_mul(ha[:, m * B:(m + 1) * B], z[:],
                                             sg[:])
                    return ha

                # fc1: 9216 -> 512 (stream weights)
                acc1 = [p_fp.tile([128, B], F32, tag=f"acc{m}", bufs=1,
                                  name=f"acc1_{m}") for m in range(4)]
                for k in range(72):
                    wt = p_f.tile([128, 4 * 128], BF16, tag="fc1w", bufs=3,
                                  name=f"fc1w_{k}")
                    nc.sync.dma_start(wt[:], fc1wp[:, k * 512:(k + 1) * 512])
                    q, h = divmod(k, 2)
                    rhs = X3q[:, h, q, :]
                    for m in range(4):
                        nc.tensor.matmul(acc1[m][:],
                                         wt[:, m * 128:(m + 1) * 128], rhs,
                                         start=(k == 0), stop=(k == 71),
                                         skip_group_check=True)
                h1a = ln_layer([a[:] for a in acc1], fc1bs, g1s, be1s, BF16, 1)

                # fc2: 512 -> 512
                acc2 = [p_fp.tile([128, B], F32, tag=f"acc{m}", bufs=1,
                                  name=f"acc2_{m}") for m in range(4)]
                for k in range(4):
                    for m in range(4):
                        nc.tensor.matmul(
                            acc2[m][:],
                            fc2ws[:, (k * 4 + m) * 128:(k * 4 + m + 1) * 128],
                            h1a[:, k * B:(k + 1) * B],
                            start=(k == 0), stop=(k == 3),
                            skip_group_check=True)
                h2a = ln_layer([a[:] for a in acc2], fc2bs, g2s, be2s, BF16, 2)

                # out: 512 -> 64
                ps_e = p_fp.tile([64, B], F32, tag="bc", bufs=2, name="ps_e")
                for k in range(4):
                    nc.tensor.matmul(ps_e[:], outws[:, k * 64:(k + 1) * 64],
                                     h2a[:, k * B:(k + 1) * B],
                                     start=(k == 0), stop=(k == 3),
                                     skip_group_check=True)
                nc.scalar.activation(embT[:], ps_e[:], AF.Identity,
                                     bias=outbs[:, 0:1])

            # ================= loss =========================================
            with (
                tc.tile_pool(name="losssb", bufs=1) as p_l,
                tc.tile_pool(name="lossps", space="PSUM", bufs=1) as p_lp,
                tc.tile_pool(name="dram", space="DRAM", bufs=1) as p_d,
            ):
                groups = [list(range(N_CORES))]

                # -- allgather target embeddings
                cc_in = p_d.tile([64, na], F32, name="cc_in")
                cc_out = p_d.tile([N_CORES * 64, na], F32, addr_space="Shared",
                                  name="cc_out")
                nc.sync.dma_start(cc_in[:], embT[:, na:2 * na])
                nc.gpsimd.collective_compute(
                    "AllGather", ALU.bypass, replica_groups=groups,
                    ins=[cc_in.opt()], outs=[cc_out.opt()])
                tT = p_l.tile([64, T], F32, tag="tT", bufs=1, name="tT")
                nc.sync.dma_start(
                    tT.rearrange("f (r j) -> f r j", r=N_CORES),
                    cc_out.rearrange("(r f) j -> f r j", r=N_CORES))

                # -- row stats of targets: nt_row[1, T], bcast ntB[128, T]
                sqt_all = p_l.tile([64, T], F32, tag="sqt_all", bufs=1,
                                   name="sqt_all")
                nc.vector.tensor_mul(sqt_all[:], tT[:], tT[:])
                nt_row = p_l.tile([1, T], F32, tag="nt_row", bufs=1,
                                  name="nt_row")
                ntB = p_l.tile([128, T], F32, tag="ntB", bufs=1, name="ntB")
                nT = (T + 511) // 512
                for n in range(nT):
                    w = min(512, T - n * 512)
                    ps_nt = p_lp.tile([1, 512], F32, tag="row", bufs=2,
                                      name=f"ps_nt{n}")
                    nc.tensor.matmul(ps_nt[:1, :w], ones[:64, :],
                                     sqt_all[:, n * 512:n * 512 + w],
                                     start=True, stop=True)
                    nc.vector.tensor_copy(nt_row[:, n * 512:n * 512 + w],
                                          ps_nt[:1, :w])
                    ps_b = p_lp.tile([128, 512], F32, tag="mm", bufs=2,
                                     name=f"ps_b{n}")
                    nc.tensor.matmul(ps_b[:, :w], onesr[:],
                                     nt_row[:, n * 512:n * 512 + w],
                                     start=True, stop=True)
                    nc.vector.tensor_copy(ntB[:, n * 512:n * 512 + w],
                                          ps_b[:, :w])

                # -- diagonal terms (all core-local)
                aT = embT[:, 0:na]
                tTl = embT[:, na:2 * na]
                prod = p_l.tile([64, na], F32, tag="dg", bufs=3, name="prod")
                sqa = p_l.tile([64, na], F32, tag="dg", bufs=3, name="sqa")
                sqtl = p_l.tile([64, na], F32, tag="dg", bufs=3, name="sqtl")
                nc.vector.tensor_mul(prod[:], aT, tTl)
                nc.vector.tensor_mul(sqa[:], aT, aT)
                nc.vector.tensor_mul(sqtl[:], tTl, tTl)
                rows3 = p_l.tile([1, 3 * na], F32, tag="rows3", bufs=1,
                                 name="rows3")
                dot_r = rows3[:, 0:na]
                na_r = rows3[:, na:2 * na]
                ntl_r = rows3[:, 2 * na:3 * na]
                for src, dst in ((prod, dot_r), (sqa, na_r), (sqtl, ntl_r)):
                    ps_d = p_lp.tile([1, 512], F32, tag="row", bufs=2,
                                     name="ps_d")
                    nc.tensor.matmul(ps_d[:1, :na], ones[:64, :], src[:],
                                     start=True, stop=True)
                    nc.vector.tensor_copy(dst, ps_d[:1, :na])
                # d2 diag -> sqrt -> sum
                r2 = p_l.tile([1, 2 * na], F32, tag="r2", bufs=1, name="r2")
                d2d = r2[:, 0:na]
                dsr = r2[:, na:2 * na]
                nc.vector.scalar_tensor_tensor(d2d, dot_r, -2.0, na_r,
                                               op0=ALU.mult, op1=ALU.add)
                nc.vector.tensor_add(d2d, d2d, ntl_r)
                nc.vector.tensor_scalar_max(d2d, d2d, 0.0)
                nc.scalar.activation(dsr, d2d, AF.Sqrt, bias=epss[:1, 1:2])
                dsum = p_l.tile([1, 1], F32, tag="dsum", bufs=1, name="dsum")
                nc.vector.tensor_reduce(dsum[:], dsr, axis=mybir.AxisListType.X,
                                        op=ALU.add)

                # -- na column vectors (na_row chunk -> [pa,1] via transpose)
                nacol = p_l.tile([128, nchunk], F32, tag="nacol", bufs=1,
                                 name="nacol")
                for ch in range(nchunk):
                    pa = min(128, na - ch * 128)
                    ps_t = p_lp.tile([128, 1], F32, tag="small", bufs=2,
                                     name=f"ps_t{ch}")
                    nc.tensor.matmul(ps_t[:pa, :],
                                     na_r[:, ch * 128:ch * 128 + pa],
                                     ident[:], start=True, stop=True,
                                     is_transpose=True)
                    nc.vector.tensor_copy(nacol[:pa, ch:ch + 1], ps_t[:pa, :])

                # -- logits rows: s = sqrt(max(na+nt-2at,0)+eps), lse
                ps_st = p_lp.tile([1, 2], F32, tag="stats", bufs=1,
                                  name="ps_st")
                for ch in range(nchunk):
                    pa = min(128, na - ch * 128)
                    lhs = embT[:, ch * 128:ch * 128 + pa]
                    s = p_l.tile([128, T], F32, tag="s", bufs=2, name=f"s{ch}")
                    for n in range(nT):
                        w = min(512, T - n * 512)
                        ps_mm = p_lp.tile([128, 512], F32, tag="mm", bufs=2,
                                          name=f"mm{ch}_{n}")
                        nc.tensor.matmul(ps_mm[:pa, :w], lhs,
                                         tT[:, n * 512:n * 512 + w],
                                         start=True, stop=True)
                        t1 = p_l.tile([128, 512], F32, tag="t1", bufs=2,
                                      name=f"t1{ch}_{n}")
                        nc.vector.scalar_tensor_tensor(
                            t1[:pa, :w], ps_mm[:pa, :w], -2.0,
                            ntB[:pa, n * 512:n * 512 + w],
                            op0=ALU.mult, op1=ALU.add)
                        nc.vector.tensor_scalar(
                            t1[:pa, :w], t1[:pa, :w], nacol[:pa, ch:ch + 1],
                            0.0, op0=ALU.add, op1=ALU.max)
                        nc.scalar.activation(s[:pa, n * 512:n * 512 + w],
                                             t1[:pa, :w], AF.Sqrt,
                                             bias=epss[:pa, 1:2])
                    sm = p_l.tile([128, 4], F32, tag="sm", bufs=2,
                                  name=f"sm{ch}")
                    smin = sm[:pa, 0:1]
                    esum = sm[:pa, 1:2]
                    lnes = sm[:pa, 2:3]
                    lse = sm[:pa, 3:4]
                    nc.vector.tensor_reduce(smin, s[:pa, :],
                                            axis=mybir.AxisListType.X,
                                            op=ALU.min)
                    scr = p_l.tile([128, T], F32, tag="scr2", bufs=1,
                                   name=f"scr{ch}")
                    nc.scalar.activation(scr[:pa, :], s[:pa, :], AF.Exp,
                                         bias=smin, scale=-1.0,
                                         accum_out=esum)
                    nc.scalar.activation(lnes, esum, AF.Ln)
                    nc.vector.tensor_sub(lse, lnes, smin)
                    pack = p_l.tile([128, 2], F32, tag="pack", bufs=2,
                                    name=f"pack{ch}")
                    nc.vector.tensor_copy(pack[:pa, 0:1], lse)
                    nc.vector.tensor_mul(pack[:pa, 1:2], lse, lse)
                    nc.tensor.matmul(ps_st[:], ones[:pa, :], pack[:pa, :],
                                     start=(ch == 0), stop=(ch == nchunk - 1),
                                     skip_group_check=True)

                # -- pack + allreduce + final scalar
                st_sb = p_l.tile([1, 4], F32, tag="st_sb", bufs=1,
                                 name="st_sb")
                nc.vector.tensor_copy(st_sb[:, 0:1], dsum[:])
                nc.vector.tensor_copy(st_sb[:, 1:3], ps_st[:])
                nc.vector.memset(st_sb[:, 3:4], 0.0)
                st_in = p_d.tile([1, 4], F32, name="st_in")
                st_out = p_d.tile([1, 4], F32, addr_space="Shared",
                                  name="st_out")
                nc.sync.dma_start(st_in[:], st_sb[:])
                nc.gpsimd.collective_compute(
                    "AllReduce", ALU.add, replica_groups=groups,
                    ins=[st_in.opt()], outs=[st_out.opt()])
                st2 = p_l.tile([1, 4], F32, tag="st2", bufs=1, name="st2")
                nc.sync.dma_start(st2[:], st_out[:])
                fin = p_l.tile([1, 3], F32, tag="fin", bufs=1, name="fin")
                nc.vector.tensor_tensor(fin[:, 0:1], st2[:, 0:1], st2[:, 1:2],
                                        op=ALU.add)
                nc.vector.scalar_tensor_tensor(fin[:, 1:2], st2[:, 2:3],
                                               C2 / C1, fin[:, 0:1],
                                               op0=ALU.mult, op1=ALU.add)
                nc.vector.tensor_scalar_mul(fin[:, 2:3], fin[:, 1:2], C1)
                nc.sync.dma_start(loss_out, fin[:, 2:3])

    nc.compile()
    return nc


# ----------------------------------------------------------------------------
# host-side input packing
# ----------------------------------------------------------------------------

def pack_weights(ins):
    """Core-independent packed weights (all bf16 except biases)."""
    out = {}
    w1 = np.asarray(ins["w1"], np.float32)
    w2 = np.asarray(ins["w2"], np.float32)
    w3 = np.asarray(ins["w3"], np.float32)
    out["w1p"] = np.ascontiguousarray(
        w1.transpose(1, 2, 3, 0).reshape(84, 9 * 64)).astype(BF)
    out["w2p"] = np.ascontiguousarray(
        w2.transpose(1, 2, 3, 0).reshape(64, 9 * 128)).astype(BF)
    out["w3p"] = np.ascontiguousarray(
        w3.reshape(2, 128, 128, 3, 3).transpose(2, 0, 3, 4, 1)
        .reshape(128, 18 * 128)).astype(BF)

    fc1w = np.asarray(ins["fc1_w"], np.float32)              # [512, 9216]
    wperm = fc1w.reshape(512, 256, 36).transpose(0, 2, 1).reshape(512, 9216)
    out["fc1wp"] = np.ascontiguousarray(
        wperm.reshape(4, 128, 72, 128).transpose(3, 2, 0, 1)
        .reshape(128, 72 * 4 * 128)).astype(BF)
    fc2w = np.asarray(ins["fc2_w"], np.float32)
    out["fc2wp"] = np.ascontiguousarray(
        fc2w.reshape(4, 128, 4, 128).transpose(3, 2, 0, 1)
        .reshape(128, 16 * 128)).astype(BF)
    outw = np.asarray(ins["out_w"], np.float32)              # [64, 512]
    out["outwp"] = np.ascontiguousarray(
        outw.reshape(64, 4, 128).transpose(2, 1, 0).reshape(128, 4 * 64)
    ).astype(BF)

    f32 = np.float32
    out["b1h"] = np.asarray(ins["b1"], f32).reshape(64, 1)
    out["b2h"] = np.asarray(ins["b2"], f32).reshape(128, 1)
    out["b3h"] = np.ascontiguousarray(
        np.asarray(ins["b3"], f32).reshape(2, 128).T)
    for src, dst in (("fc1_b", "fc1bh"), ("ln1_g", "g1h"), ("ln1_b", "be1h"),
                     ("fc2_b", "fc2bh"), ("ln2_g", "g2h"), ("ln2_b", "be2h")):
        out[dst] = np.ascontiguousarray(
            np.asarray(ins[src], f32).reshape(4, 128).T)
    out["outbh"] = np.asarray(ins["out_b"], f32).reshape(64, 1)
    out["onesh"] = np.ones((128, 1), f32)
    eps = np.zeros((128, 2), f32)
    eps[:, 0] = LN_EPS
    eps[:, 1] = DIST_EPS
    out["epsh"] = eps
    out["onesrh"] = np.ones((1, 128), f32)
    out["identh"] = np.ones((1, 1), f32)
    return out


def pack_obs(obs, B=B_CORE):
    """Per-core padded/transposed obs: list of [84, B*169] bf16."""
    obs = np.asarray(obs, np.float32)
    Btot = obs.shape[0]
    mid = Btot // 2
    na = B // 2
    x = (obs / 255.0).astype(BF)
    xp = np.zeros((Btot, 84, 13, 13), BF)
    xp[:, :, 1:12, 1:12] = x
    per_core = []
    for c in range(N_CORES):
        idx = np.r_[na * c:na * (c + 1), mid + na * c:mid + na * (c + 1)]
        oc = np.ascontiguousarray(
            xp[idx].transpose(1, 0, 2, 3).reshape(84, B * 169))
        per_core.append(oc)
    return per_core


def make_in_maps(ins, B=B_CORE):
    w = pack_weights(ins)
    per_core_obs = pack_obs(ins["obs"], B)
    return [{**w, "obs_t": per_core_obs[c]} for c in range(N_CORES)]


# ----------------------------------------------------------------------------
# entry point
# ----------------------------------------------------------------------------

_CACHE = {}


def _get_program(B=B_CORE, nb=NB):
    key = (B, nb)
    if key not in _CACHE:
        _CACHE[key] = build_program(B, nb)
    return _CACHE[key]


def run(ins, trace=False, B=B_CORE, nb=NB):
    nc = _get_program(B, nb)
    in_maps = make_in_maps(ins, B)
    res = run_bass_kernel_spmd(nc, in_maps, list(range(N_CORES)), trace=trace)
    val = np.float32(res.results[0]["loss_out"][0, 0])
    return val, res


def kernel(**inputs):
    val, _ = run(inputs)
    return np.asarray(val, dtype=np.float32).reshape(())
